# revision 70
# baseline (speedup 1.0000x reference)
"""Ragged masked-attention TRN2 kernel (nn_AttentionBase, B=16 Q=K=D=1024 fp32).

Sharding: data-parallel over batch, 8 cores, one SPMD program. The program
is a static schedule of per-q-tile "tasks" computed at runtime from the
actual query_lens/key_lens: rows beyond query_len and keys beyond key_len
contribute nothing to the output (softmax weight 0 / output row 0), so the
schedule only covers q-tiles < ceil(q_len/128), with per-task k-windows
(padded to >=2 chunks for full-rate fp32r matmuls).

Structure: phases (one double-buffered K/V residency each) x slots
(disjoint k-chunk ranges within a phase's buffer). Every core runs the
same task list; a core hosts at most one batch per slot, and a batch may
be split across several cores' cells of the same slot. A simulated-
annealing search over slot layouts/splits minimizes a calibrated chain
model (serial DMA chain vs PE progression incl. pstate ramp resets);
all input DMAs are emitted on one queue in task-need order so bytes land
just in time. Host packs Q^T tiles / K^T / V / bias / qmask per (core,
phase, task) and scatters bf16 task outputs into the full fp32 output.

Numerics: scores fp32r (PE full rate), softmax stats fp32, bias rows /
exp-weights / V / output bf16. Measured rel err 5.9e-3 vs the 2e-2 gate.
Baseline 271 us -> this kernel ~91 us per core (cost-model timeline).
"""

import sys

sys.path.insert(0, "/opt/trn_rl_repo")

from itertools import combinations

import numpy as np
import ml_dtypes

P = 128
N_CORES = 8
SEQ = 1024
D = 1024
NCH = SEQ // P  # 8 chunks of 128 along any 1024 dim
NEG = np.float32(-1e30)

_CACHE = {}


# ---------------------------------------------------------------- schedule
#
# Schedule model: each phase loads one K/V buffer of W k-chunks, laid out as
# disjoint "slots" (offset, width). A slot has a task-window profile; every
# core runs every task. A core may host at most one batch per slot (its K/V
# at the slot offset); a batch may be split across several cores' cells of
# the same slot (each cell computes a subset of its q-tiles against the
# batch's full K). Cost per core is static: sum over tasks of the window
# size. Search minimizes an estimated wall time (PE vs DMA roofline).

W_MAX = 10  # max K chunks resident per phase (SBUF budget)


def _slot_profile(members, qm, km):
    """members: {batch: ncells}. Task profile (desc): profile[t] = max km
    over members whose per-cell tile count exceeds t."""
    tiles = {b: -(-qm[b] // n) for b, n in members.items()}
    L = max(tiles.values())
    return [max(km[b] for b in members if tiles[b] > t) for t in range(L)]


NS_PER_MB = 2900.0  # DMA chain rate (344 GB/s aggregate)
UNIT_NS = 1000.0  # PE ns per (qtile x kchunk) unit at full pstate incl. overheads
TASK_NS = 350.0  # per-task pipeline bubble
ISSUE_NS = 1800.0  # DMA issue ramp before first transfer
TAIL_NS = 6000.0  # drain + last store


def _eval_state(state, qm, km):
    """Mini-sim: DMA transfers form one serial chain (K_s, V_s, per-slot Qs,
    next phase...); each slot's tasks start after its K (+V slightly before
    their tail) has landed and the PE is free."""
    units = tasks = 0
    phases = []
    for ph in state:
        W = 0
        slots = []
        for s in ph:
            if not s["m"]:
                continue
            if sum(s["m"].values()) > 8 or any(km[b] > s["w"] for b in s["m"]):
                return None
            prof = _slot_profile(s["m"], qm, km)
            units += sum(prof)
            tasks += len(prof)
            W += s["w"]
            slots.append((s["w"], sorted(prof)))
        if W > W_MAX or W == 0:
            return None
        slots.sort()
        phases.append(slots)

    def chain_wall(ordered):
        dma_t = ISSUE_NS
        pe_t = ISSUE_NS
        ramp_end = 1e18  # mid-pstate (2x cycles) until 3us continuous busy
        out_mb = 0.0
        per_dma = 680.0  # per-transfer issue/HWDGE overhead in the chain
        for pidx, slots in enumerate(ordered):
            for sidx, (w, prof) in enumerate(slots):
                dma_t += w * 0.5 * NS_PER_MB + per_dma  # K f32r
                k_done = dma_t
                dma_t += w * 0.25 * NS_PER_MB + per_dma  # V bf16
                v_done = dma_t
                for kw in prof:
                    dma_t += 0.5 * NS_PER_MB + per_dma  # Q tile f32r
                    q_done = dma_t
                    start = max(pe_t, k_done, v_done - 2000.0, q_done - 1000.0)
                    if start - pe_t > 200.0:
                        ramp_end = start + 3000.0  # stall resets the PE pstate
                    cost = kw * UNIT_NS + TASK_NS
                    if start < ramp_end:
                        cost += min(ramp_end - start, cost)  # 2x inside ramp
                    pe_t = start + cost
                    out_mb += 0.25
        return max(pe_t, dma_t + out_mb * NS_PER_MB * 0.6) + TAIL_NS

    from itertools import permutations

    best = None
    for perm in permutations(range(len(phases))):
        wall = chain_wall([phases[i] for i in perm])
        if best is None or wall < best[0]:
            best = (wall, perm)
    return best[0], units, tasks, best[1]


def _search_schedule(qm, km, iters=80000, seed=0):
    """Simulated-annealing search over slot layouts/assignments."""
    import math
    import random

    B = len(qm)
    rng = random.Random(seed)

    # init: best 8/8 partition, one slot per phase (variant-c equivalent)
    def profile_cost(group):
        T = max(qm[b] for b in group)
        return sum(max(km[b] for b in group if qm[b] > t) for t in range(T))

    best_part = None
    allb = list(range(B))
    for g0 in combinations(allb, B // 2):
        g1 = tuple(b for b in allb if b not in g0)
        c = profile_cost(g0) + profile_cost(g1)
        if best_part is None or c < best_part[0]:
            best_part = (c, g0, g1)
    _, g0, g1 = best_part
    state = [
        [{"w": max(km[b] for b in g), "m": {b: 1 for b in g}}] for g in (g0, g1)
    ]

    cur = _eval_state(state, qm, km)
    assert cur is not None
    best = (cur[0], [[{"w": s["w"], "m": dict(s["m"])} for s in ph] for ph in state])

    def copy_state(st):
        return [[{"w": s["w"], "m": dict(s["m"])} for s in ph] for ph in st]

    temp0 = 3000.0
    for it in range(iters):
        temp = temp0 * (1.0 - it / iters) + 1.0
        cand = copy_state(state)
        # pick a random batch placement
        locs = [
            (pi, si, b)
            for pi, ph in enumerate(cand)
            for si, s in enumerate(ph)
            for b in s["m"]
        ]
        pi, si, b = locs[rng.randrange(len(locs))]
        mv = rng.random()
        if mv < 0.45:
            # relocate batch to another slot (possibly new, possibly in a
            # brand-new third phase)
            n = cand[pi][si]["m"].pop(b)
            tpi = rng.randrange(len(cand))
            tph = cand[tpi]
            choices = [s for s in tph if s["w"] >= km[b]] + ["new"]
            tgt = choices[rng.randrange(len(choices))]
            if tgt == "new":
                tph.append({"w": km[b], "m": {b: n}})
            else:
                tgt["m"][b] = n
        elif mv < 0.8:
            # change split factor
            n = cand[pi][si]["m"][b]
            cand[pi][si]["m"][b] = max(1, n + rng.choice([-1, 1]))
        else:
            # change slot width
            s = cand[pi][si]
            s["w"] = max(max(km[x] for x in s["m"]), s["w"] + rng.choice([-1, 1]))
        for ph in cand:
            ph[:] = [s for s in ph if s["m"]]
        cand = [ph for ph in cand if ph]
        if not cand:
            continue
        r = _eval_state(cand, qm, km)
        if r is None:
            continue
        if r[0] <= cur[0] or rng.random() < math.exp((cur[0] - r[0]) / temp):
            state, cur = cand, r
            if r[0] < best[0]:
                best = (r[0], copy_state(state))
    return best[1]


def _make_schedule(query_lens, key_lens):
    B = len(query_lens)
    qm = [max(1, -(-int(q) // P)) for q in query_lens]
    km = [max(2, -(-int(k) // P)) for k in key_lens]  # >=2 keeps matmul F>=256

    global W_MAX
    wmax_configs = sorted({max(w, max(km)) for w in (8, 9, 10)})
    if "seed" in _CACHE:  # sweep override: single config
        wmax_configs = [max(W_MAX, max(km))]
        seeds = [_CACHE["seed"]]
    else:
        seeds = list(range(8))
    sched_key = ("sched", tuple(qm), tuple(km))
    if sched_key in _CACHE:
        return _CACHE[sched_key]
    best = None
    for wm in wmax_configs:
        W_MAX = wm
        for sd in seeds:
            st = _search_schedule(qm, km, seed=sd)
            rr = _eval_state(st, qm, km)
            if rr is not None and (best is None or rr[0] < best[0][0]):
                best = (rr, st)
    r, state = best
    _CACHE["last_eval"] = r
    # phase order chosen by the evaluator's chain model
    perm = r[3]
    import os
    if os.environ.get("PHASE_FLIP"):
        perm = tuple(reversed(perm))
    state = [state[i] for i in perm]

    phases = []
    for pi, ph_slots in enumerate(state):
        last_phase = False
        ph_slots.sort(key=lambda s: s["w"])
        tasks = []
        slots = []
        assign = [[] for _ in range(N_CORES)]
        off = 0
        for si, s in enumerate(ph_slots):
            prof = _slot_profile(s["m"], qm, km)
            base = len(tasks)
            # ascending task order (small windows first) except the very
            # last slot of the last phase, which runs descending so the
            # pipeline tail ends on the smallest window
            last_slot = last_phase and si == len(ph_slots) - 1
            order = prof if last_slot else prof[::-1]
            tasks.extend((off, kw) for kw in order)
            slots.append((off, s["w"], len(prof)))
            L = len(prof)
            core = 0
            for b, n in sorted(s["m"].items()):
                # distribute qm[b] tiles over n cells (cores)
                tiles = qm[b]
                per = -(-tiles // n)
                done = 0
                for cell in range(n):
                    cnt = min(per, tiles - done)
                    if cnt <= 0:
                        break
                    # cell uses the cnt largest-window tasks: last cnt in
                    # ascending order, first cnt in descending order
                    if last_slot:
                        tlist = [(base + j, done + j) for j in range(cnt)]
                    else:
                        tlist = [(base + L - cnt + j, done + j) for j in range(cnt)]
                    assign[core].append((b, off, tlist))
                    core += 1
                    done += cnt
            off += s["w"]
        phases.append({"w": off, "tasks": tasks, "slots": slots, "assign": assign})
    skeleton = tuple(
        (ph["w"], tuple(ph["tasks"]), tuple(ph["slots"])) for ph in phases
    )
    result = (phases, skeleton, qm, km)
    _CACHE[("sched", tuple(qm), tuple(km))] = result
    return result


# ---------------------------------------------------------------- program


def _build_nc(skeleton):
    import concourse.bass as bass  # noqa: F401
    import concourse.mybir as mybir
    import concourse.tile as tile
    from concourse import bacc
    from concourse.masks import make_identity

    f32 = mybir.dt.float32
    f32r = mybir.dt.float32r
    bf16 = mybir.dt.bfloat16
    X = mybir.AxisListType.X
    Exp = mybir.ActivationFunctionType.Exp

    phases = [
        {"w": w, "tasks": list(tasks), "slots": list(slots)}
        for w, tasks, slots in skeleton
    ]
    # per-parity K/V buffer widths (phase p uses buffer p % 2)
    wpar = [
        max((ph["w"] for ph in phases[par::2]), default=0) for par in range(2)
    ]
    kwmax = max(kw for ph in phases for _, kw in ph["tasks"])
    tmax = max(len(ph["tasks"]) for ph in phases)

    nc = bacc.Bacc("TRN2", target_bir_lowering=False, debug=False)

    for p, ph in enumerate(phases):
        w, T = ph["w"], len(ph["tasks"])
        ph["k_d"] = nc.dram_tensor(f"k{p}", [NCH, P, w * P], f32r, kind="ExternalInput")
        ph["v_d"] = nc.dram_tensor(f"v{p}", [w, P, D], bf16, kind="ExternalInput")
        ph["q_d"] = nc.dram_tensor(f"q{p}", [T, NCH, P, P], f32r, kind="ExternalInput")
        ph["bias_d"] = nc.dram_tensor(
            f"bias{p}", [1, T, kwmax * P], bf16, kind="ExternalInput"
        )
        ph["qm_d"] = nc.dram_tensor(f"qm{p}", [P, T], f32, kind="ExternalInput")
        ph["out_d"] = nc.dram_tensor(f"out{p}", [T, P, D], bf16, kind="ExternalOutput")

    def score_chunks(kw):
        """Split kw*128 score columns into PSUM chunks, each <=512 and
        (for fp32r full rate) >=256 columns."""
        n = kw * P
        if n <= 512:
            return [(0, n)]
        if n <= 896:
            h = (kw // 2) * P
            return [(0, h), (h, n - h)]
        return [(0, 512), (512, n - 512)]

    with tile.TileContext(nc) as tc:
        with (
            tc.tile_pool(name="const", bufs=1) as const_pool,
            tc.tile_pool(name="kv", bufs=1) as kv_pool,
            tc.tile_pool(name="qs", bufs=5) as qs_pool,
            tc.tile_pool(name="w", bufs=2) as w_pool,
            tc.tile_pool(name="wt", bufs=2) as wt_pool,
            tc.tile_pool(name="ob", bufs=2) as ob_pool,
            tc.tile_pool(name="stat", bufs=6) as stat,
            tc.tile_pool(name="ps_s", bufs=3, space="PSUM") as ps_s,
            tc.tile_pool(name="ps_t", bufs=3, space="PSUM") as ps_t,
            tc.tile_pool(name="ps_o", bufs=1, space="PSUM") as ps_o,
        ):
            identity_f32 = const_pool.tile([P, P], f32, tag="ident32")
            make_identity(nc, identity_f32)
            identity = const_pool.tile([P, P], bf16, tag="ident")
            nc.vector.tensor_copy(identity[:], identity_f32[:])
            ones_f32 = const_pool.tile([1, P], f32, tag="ones32")
            nc.gpsimd.memset(ones_f32[:], 1.0)
            ones = const_pool.tile([1, P], bf16, tag="ones")
            nc.vector.tensor_copy(ones[:], ones_f32[:])

            # per-phase-parity K/V/bias/qmask tiles (double buffer)
            kt = {}
            vt = {}
            bt = {}
            qmt = {}
            for par in range(2):
                if wpar[par] == 0:
                    continue
                kt[par] = kv_pool.tile(
                    [P, NCH, wpar[par] * P], f32r, tag=f"k{par}", name=f"k{par}"
                )
                vt[par] = kv_pool.tile(
                    [P, wpar[par], D], bf16, tag=f"v{par}", name=f"v{par}"
                )
                bt[par] = kv_pool.tile(
                    [1, tmax, kwmax * P], bf16, tag=f"b{par}", name=f"b{par}"
                )
                qmt[par] = kv_pool.tile([P, tmax], f32, tag=f"m{par}", name=f"m{par}")

            def load_misc(p):
                ph, par = phases[p], p % 2
                T = len(ph["tasks"])
                nc.gpsimd.dma_start(bt[par][:, :T], ph["bias_d"].ap())
                nc.gpsimd.dma_start(qmt[par][:, :T], ph["qm_d"].ap())

            def load_slot_k(p, off, w):
                par = p % 2
                nc.sync.dma_start(
                    kt[par][:, :, off * P : (off + w) * P],
                    phases[p]["k_d"].ap()[:, :, off * P : (off + w) * P].rearrange(
                        "d p c -> p d c"
                    ),
                )

            def load_slot_v(p, off, w):
                par = p % 2
                nc.sync.dma_start(
                    vt[par][:, off : off + w],
                    phases[p]["v_d"].ap()[off : off + w].rearrange("j p c -> p j c"),
                )

            def load_q(p, t):
                qtile = qs_pool.tile([P, NCH, P], f32r, tag="q", name=f"q{p}_{t}")
                nc.sync.dma_start(
                    qtile[:], phases[p]["q_d"].ap()[t].rearrange("d p c -> p d c")
                )
                return qtile

            stageb = {}

            def emit_stage_a(p, t, qtile):
                ph, par = phases[p], p % 2
                off, kw = ph["tasks"][t]
                chunks = score_chunks(kw)
                w_sb = w_pool.tile([P, kwmax * P], bf16, tag="w", name="w")
                nm2 = stat.tile([P, 2], f32, tag="nm2", name="nm2")
                rs = stat.tile([P, 2], f32, tag="rs", name="rs")
                pss = []
                for i, (c0, sz) in enumerate(chunks):
                    ps = ps_s.tile([P, 512], f32, tag="s", name=f"s{i}")
                    for d in range(NCH):
                        nc.tensor.matmul(
                            ps[:, :sz],
                            qtile[:, d],
                            kt[par][:, d, off * P + c0 : off * P + c0 + sz],
                            start=(d == 0),
                            stop=False,
                        )
                    nc.tensor.matmul(
                        ps[:, :sz],
                        ones[:],
                        bt[par][:, t, c0 : c0 + sz],
                        start=False,
                        stop=True,
                    )
                    nc.vector.reduce_max(nm2[:, i : i + 1], ps[:, :sz], axis=X, negate=True)
                    pss.append(ps)
                if len(chunks) == 2:
                    negmax = stat.tile([P, 1], f32, tag="negmax", name="negmax")
                    nc.vector.tensor_tensor(
                        negmax[:], nm2[:, 0:1], nm2[:, 1:2], mybir.AluOpType.min
                    )
                else:
                    negmax = nm2[:, 0:1]
                for i, (c0, sz) in enumerate(chunks):
                    nc.scalar.activation(
                        w_sb[:, c0 : c0 + sz],
                        pss[i][:, :sz],
                        Exp,
                        bias=negmax if len(chunks) == 2 else nm2[:, 0:1],
                        accum_out=rs[:, i : i + 1],
                    )
                if len(chunks) == 2:
                    rsum = stat.tile([P, 1], f32, tag="rsum", name="rsum")
                    nc.vector.tensor_tensor(
                        rsum[:], rs[:, 0:1], rs[:, 1:2], mybir.AluOpType.add
                    )
                else:
                    rsum = rs[:, 0:1]
                rcp = stat.tile([P, 1], f32, tag="rcp", name="rcp")
                nc.vector.reciprocal(rcp[:], rsum)
                scal = stat.tile([P, 1], f32, tag="scal", name="scal")
                nc.vector.tensor_tensor(
                    scal[:], rcp[:], qmt[par][:, t : t + 1], mybir.AluOpType.mult
                )
                stageb[(p, t)] = (w_sb, scal)

            def emit_stage_b(p, t, last):
                ph, par = phases[p], p % 2
                off, kw = ph["tasks"][t]
                w_sb, scal = stageb.pop((p, t))
                wts = []
                for j in range(kw):
                    pst = ps_t.tile([P, P], bf16, tag="pst", name="pst")
                    nc.tensor.transpose(
                        pst[:], w_sb[:, j * P : (j + 1) * P], identity[:]
                    )
                    wtj = wt_pool.tile([P, P], bf16, tag=f"wt{j}", name=f"wt{j}")
                    nc.any.tensor_copy(wtj[:], pst[:])
                    wts.append(wtj)
                out_sb = ob_pool.tile([P, D], bf16, tag="outsb")
                for n2 in range(2):
                    po = ps_o.tile([P, 512], f32, tag=f"o{n2}", name=f"o{n2}")
                    for j in range(kw):
                        nc.tensor.matmul(
                            po[:],
                            wts[j][:],
                            vt[par][:, off + j, n2 * 512 : (n2 + 1) * 512],
                            start=(j == 0),
                            stop=(j == kw - 1),
                        )
                    nc.any.tensor_scalar_mul(
                        out_sb[:, n2 * 512 : (n2 + 1) * 512], po[:], scal[:]
                    )
                    if last:
                        # final task: store each half as soon as it is
                        # scaled, shortening the kernel tail
                        nc.sync.dma_start(
                            ph["out_d"].ap()[t, :, n2 * 512 : (n2 + 1) * 512],
                            out_sb[:, n2 * 512 : (n2 + 1) * 512],
                        )
                if not last:
                    nc.gpsimd.dma_start(ph["out_d"].ap()[t], out_sb[:])

            # flat task list with cross-phase software pipeline. All input
            # DMAs go on ONE queue in need order (K_s, V_s, then the slot's
            # Q tiles), drained with a 2-task Q lookahead, so the serial DMA
            # chain delivers bytes just in time.
            flat = [(p, t) for p, ph in enumerate(phases) for t in range(len(ph["tasks"]))]
            # chain order per slot: K, first two Q tiles, V, remaining Qs —
            # the slot's first tasks aren't starved behind its V transfer
            dma_order = []
            for p, ph in enumerate(phases):
                t0 = 0
                for off, w, ntasks in ph["slots"]:
                    qs = [("q", p, t) for t in range(t0, t0 + ntasks)]
                    dma_order.append(("k", p, off, w))
                    dma_order.extend(qs[:2])
                    dma_order.append(("v", p, off, w))
                    dma_order.extend(qs[2:])
                    t0 += ntasks
            qtiles = {}
            cursor = 0

            def drain_until(p, t):
                nonlocal cursor
                while cursor < len(dma_order):
                    e = dma_order[cursor]
                    cursor += 1
                    if e[0] == "k":
                        load_slot_k(e[1], e[2], e[3])
                    elif e[0] == "v":
                        load_slot_v(e[1], e[2], e[3])
                    else:
                        qtiles[(e[1], e[2])] = load_q(e[1], e[2])
                        if (e[1], e[2]) == (p, t):
                            break
                # peek-emit any immediately following K/V entries so the
                # next slot's buffers start transferring as early as possible
                while cursor < len(dma_order) and dma_order[cursor][0] != "q":
                    e = dma_order[cursor]
                    cursor += 1
                    if e[0] == "k":
                        load_slot_k(e[1], e[2], e[3])
                    else:
                        load_slot_v(e[1], e[2], e[3])

            LOOKAHEAD = 5
            for p in range(len(phases)):
                load_misc(p)
            for i in range(min(LOOKAHEAD, len(flat))):
                drain_until(*flat[i])
            for i, (p, t) in enumerate(flat):
                emit_stage_a(p, t, qtiles.pop((p, t)))
                if i + LOOKAHEAD < len(flat):
                    drain_until(*flat[i + LOOKAHEAD])
                if i >= 1:
                    pp, tt = flat[i - 1]
                    emit_stage_b(pp, tt, last=False)
            pp, tt = flat[-1]
            emit_stage_b(pp, tt, last=True)
    nc.compile()
    return nc


def _get_nc(skeleton=None):
    if skeleton is None:
        skeleton = _CACHE.get("last_skeleton")
        assert skeleton is not None, "no schedule computed yet"
    if ("nc", skeleton) not in _CACHE:
        _CACHE[("nc", skeleton)] = _build_nc(skeleton)
    _CACHE["last_skeleton"] = skeleton
    return _CACHE[("nc", skeleton)]


# ---------------------------------------------------------------- host side


def _prep_in_maps(phases, qm, km, queries, keys, values, query_lens, key_lens):
    kwmax = max(kw for ph in phases for _, kw in ph["tasks"])
    in_maps = []
    for c in range(N_CORES):
        m = {}
        for p, ph in enumerate(phases):
            w, T = ph["w"], len(ph["tasks"])
            kbuf = np.zeros((NCH, P, w * P), np.float32)
            vbuf = np.zeros((w, P, D), ml_dtypes.bfloat16)
            qbuf = np.zeros((T, NCH, P, P), np.float32)
            bbuf = np.zeros((1, T, kwmax * P), ml_dtypes.bfloat16)
            qmbuf = np.zeros((P, T), np.float32)
            for b, off, tasks in ph["assign"][c]:
                kmb, qlb, klb = km[b], int(query_lens[b]), int(key_lens[b])
                kT = keys[b].T.reshape(NCH, P, SEQ)  # [d, p, k]
                kbuf[:, :, off * P : (off + kmb) * P] = kT[:, :, : kmb * P]
                vbuf[off : off + kmb] = (
                    values[b].reshape(NCH, P, D)[:kmb].astype(ml_dtypes.bfloat16)
                )
                qT = queries[b].T.reshape(NCH, P, NCH, P)  # [d, p, m, c]
                for t, qt in tasks:
                    toff, kw = ph["tasks"][t]
                    assert toff == off and kw >= kmb
                    qbuf[t] = qT[:, :, qt, :]
                    bbuf[0, t, : kw * P] = np.where(
                        np.arange(kw * P) < klb, np.float32(0.0), NEG
                    ).astype(ml_dtypes.bfloat16)
                    qmbuf[:, t] = (qt * P + np.arange(P)) < qlb
            m[f"k{p}"] = kbuf
            m[f"v{p}"] = vbuf
            m[f"q{p}"] = qbuf
            m[f"bias{p}"] = bbuf
            m[f"qm{p}"] = qmbuf
        in_maps.append(m)
    return in_maps


def _run(inputs, trace=False, trace_kwargs=None):
    from concourse.bass_utils import run_bass_kernel_spmd

    queries = np.asarray(inputs["queries"], dtype=np.float32)
    keys = np.asarray(inputs["keys"], dtype=np.float32)
    values = np.asarray(inputs["values"], dtype=np.float32)
    query_lens = np.asarray(inputs["query_lens"]).astype(np.int64)
    key_lens = np.asarray(inputs["key_lens"]).astype(np.int64)
    B = queries.shape[0]
    assert B == 2 * N_CORES

    phases, skeleton, qm, km = _make_schedule(query_lens, key_lens)
    in_maps = _prep_in_maps(
        phases, qm, km, queries, keys, values, query_lens, key_lens
    )

    nc = _get_nc(skeleton)
    kwargs = {}
    if trace:
        kwargs["trace"] = True
        if trace_kwargs:
            kwargs.update(trace_kwargs)
    try:
        res = run_bass_kernel_spmd(nc, in_maps, core_ids=list(range(N_CORES)), **kwargs)
    except Exception:
        # transient device wedges usually clear on the next attempt
        import time

        time.sleep(5)
        res = run_bass_kernel_spmd(nc, in_maps, core_ids=list(range(N_CORES)), **kwargs)

    out = np.zeros((B, SEQ, D), np.float32)
    for c in range(N_CORES):
        for p, ph in enumerate(phases):
            o = res.results[c][f"out{p}"]
            for b, off, tasks in ph["assign"][c]:
                for t, qt in tasks:
                    out[b, qt * P : (qt + 1) * P, :] = o[t].astype(np.float32)
    return out, res


def kernel(**inputs) -> np.ndarray:
    out, _ = _run(inputs, trace=False)
    return out
